# revision 17
# baseline (speedup 1.0000x reference)
"""Trainium2 Bass kernel for a 3-layer GCN + mean-pool + MLP + softmax.

Reference computation (N=16384 nodes, dense adjacency):
    Ahat = D^-1/2 (A + I) D^-1/2
    H0 = X;  H_{l+1} = relu(Ahat @ (H_l @ W_l) + b_l)   l = 0,1,2
    g = mean(H3, axis=0);  h1 = elu(g @ Wh1 + bh1)
    logits = h1 @ Wh2 + bh2;  probs = softmax(logits)

Distribution (8 NeuronCores, 1D node/row parallel), v6 schedule:
  - Host folds degree normalization into the adjacency (fp8, ASCALE) and
    ALSO folds W0 into the layer-0 stationary (stat0 = X @ W0, fp8), so
    all three layers have the identical stream structure:
    acc = (ASCALE Ahat) @ stat;  h = relu(acc + ASCALE b).
  - Each layer computes its 2048 output columns in FOUR chunks
    (1024, 512, 256, 256 cols).  Each chunk contracts over the full
    j=16384 stream, tiled in groups: chunk0 8x[128,16,1024] (2MB),
    chunk1 4x[128,32,512] (2MB), chunks2/3 4x[128,32,256] (1MB) each.
  - The next layer's stationary is gathered in four matching pieces:
    AllGather(piece q) fires when chunk q finalizes (50/75/87.5/100% of
    the layer).  Contraction j-order is a permutation placing piece-p
    rows in decreasing-stripe-count order (64/32/16/16 stripes) inside
    EVERY chunk, so piece p is first needed at 50/50/75/87.5% through
    chunk 0 of the next layer.  All pieces get >=26us of cover over the
    measured AG chain (~20-25us incl. rank skew), so collectives hide
    under the adjacency stream.
  - Streaming: one 3-deep 2MB ring on the SP (nc.sync) HWDGE ring.
    9 of 20 groups (14MB) are cached in SBUF across layers (fills
    during layer 0): 3x2MB chunk0, 2x2MB chunk1, 2x1MB chunk2, 2x1MB
    chunk3 -> layers 1/2 stream 18MB each.
  - Collective-input DMAs ride SWDGE (gpsimd) so their completion
    semaphores never alias the HWDGE lanes carrying 2MB stream DMAs.
    Stat loads ride the ACT ring (one DMA per piece, rank-major
    rearrange).  Final pool collective in/out ride ACT (stream is over,
    no aliasing risk) to cut the ~7us SWDGE small-DMA latency.
  - Mean pool: per-chunk partial reductions into one [64,4] tile, one
    combine reduce, AllGather of 256B partials + local sum.
  - HAM keep-alive: tiny dummy matmuls after each group keep the PE
    clock-gate at 2.4GHz through per-group DMA slack.
"""

import numpy as np
import ml_dtypes

N = 16384
NCORES = 8
ROWS = N // NCORES          # 2048 output nodes per core
P = 128
DIMS = [64, 32, 48, 64]     # feature dims: in, after l0, l1, l2
CW = (1024, 512, 256, 256)  # output-chunk widths
IOFF = (0, 1024, 1536, 1792)
NGQ = (16, 8, 4, 4)         # groups per chunk (32 total, 1MB each)
NTQ = (8, 16, 32, 32)       # j-stripes per group
NU = (8, 4, 2, 2)           # u-tiles per piece (pieces == chunks)
ASCALE = 16.0               # fp8 range helper for Ahat
ABUFS = 6                   # streamed groups in flight
# stream positions (0..31) cached in SBUF across layers (16 x 1MB)
CACHED_IDS = (0, 2, 3, 6, 9, 10, 12, 15,   # chunk0 (of 16)
              16, 18, 20, 22,              # chunk1 (of 8)
              24, 26,                      # chunk2 (of 4)
              28, 30)                      # chunk3 (of 4)

_nc_cache = None


def _stripe_rv(sp):
    """Stripe index -> (piece, r, u).  Pieces own 64/32/16/16 stripes:
    piece0: sp<64 (u 0-7), piece1: 64-95 (u 8-11), piece2: 96-111
    (u 12-13), piece3: 112-127 (u 14-15)."""
    if sp < 64:
        return 0, sp // 8, sp % 8
    if sp < 96:
        sb = sp - 64
        return 1, sb // 4, 8 + sb % 4
    if sp < 112:
        sb = sp - 96
        return 2, sb // 2, 12 + sb % 2
    sb = sp - 112
    return 3, sb // 2, 14 + sb % 2


def _build_nc():
    from concourse import bacc, mybir, tile

    dt = mybir.dt
    F32 = dt.float32
    F8 = dt.float8e4
    BF16 = dt.bfloat16
    AF = mybir.ActivationFunctionType
    OP = mybir.AluOpType
    DR = mybir.MatmulPerfMode.DoubleRow

    nc = bacc.Bacc(
        "TRN2", target_bir_lowering=False, debug=False, num_devices=NCORES
    )

    # adjacency pre-tiled on host (see kernel())
    a_c0 = nc.dram_tensor(
        "a_c0", [NGQ[0], P, NTQ[0], CW[0]], F8, kind="ExternalInput"
    )
    a_c1 = nc.dram_tensor(
        "a_c1", [NGQ[1], P, NTQ[1], CW[1]], F8, kind="ExternalInput"
    )
    a_c23 = nc.dram_tensor(
        "a_c23", [NGQ[2] + NGQ[3], P, NTQ[2], CW[2]], F8, kind="ExternalInput"
    )
    # layer-0 stationary = X@W0 (fp8, host-tiled): [p, r, u, c],
    # node j = r*2048 + u*128 + p
    s0 = nc.dram_tensor(
        "s0", [P, NCORES, 16, DIMS[1]], F8, kind="ExternalInput"
    )
    # all small f32 params packed in one block (one DMA, one sem lane):
    # cols: b0 | b1 | b2 | w1f | w2f | wh1 | bh1 | wh2 | bh2
    CPK_K = 3 + DIMS[2] + DIMS[3] + 32 + 1 + 2 + 1
    cpk_d = nc.dram_tensor("cpk", [DIMS[0], CPK_K], F32, kind="ExternalInput")
    logits_o = nc.dram_tensor("logits", [2, 1], F32, kind="ExternalOutput")
    probs_o = nc.dram_tensor("probs", [2, 1], F32, kind="ExternalOutput")

    rg = [list(range(NCORES))]

    def group_meta(gp):
        """Stream position -> (dram handle, slot)."""
        if gp < 16:
            return a_c0, gp
        if gp < 24:
            return a_c1, gp - 16
        return a_c23, gp - 24

    with tile.TileContext(nc) as tc:
        with (
            tc.tile_pool(name="const", bufs=1) as const,
            tc.tile_pool(name="apool", bufs=ABUFS) as apool,
            tc.tile_pool(name="cpool", bufs=len(CACHED_IDS)) as cpool,
            tc.tile_pool(name="spool", bufs=2) as spool,
            tc.tile_pool(name="hpool", bufs=2) as hpool,
            tc.tile_pool(name="ypool", bufs=2) as ypool,
            tc.tile_pool(name="smal", bufs=1) as smal,
            tc.tile_pool(name="accp", bufs=1, space="PSUM") as accp,
            tc.tile_pool(name="psml", bufs=2, space="PSUM") as psml,
            tc.tile_pool(name="psmlp", bufs=1, space="PSUM") as psmlp,
            tc.tile_pool(name="dram", bufs=1, space="DRAM") as dram,
        ):
            # ---- layer-0 stationary (fp8, host-tiled) ----
            stat0 = const.tile([P, NCORES, 16, DIMS[1]], F8, name="sx")
            nc.scalar.dma_start(stat0[:], s0.ap())

            # ---- constants: ONE packed DMA (one sem lane), then views ----
            CPK_K = 3 + DIMS[2] + DIMS[3] + 32 + 1 + 2 + 1
            cpk = const.tile([DIMS[0], CPK_K], F32, name="cpk")
            nc.scalar.dma_start(cpk[:], cpk_d.ap())
            b_sb = [cpk[: DIMS[l + 1], l : l + 1] for l in range(3)]
            c0 = 3
            w1f = cpk[: DIMS[1], c0 : c0 + DIMS[2]]
            c0 += DIMS[2]
            w2f = cpk[: DIMS[2], c0 : c0 + DIMS[3]]
            c0 += DIMS[3]
            wh1_sb = cpk[: DIMS[3], c0 : c0 + 32]
            c0 += 32
            bh1_sb = cpk[:32, c0 : c0 + 1]
            c0 += 1
            wh2_sb = cpk[:32, c0 : c0 + 2]
            c0 += 2
            bh2_sb = cpk[:2, c0 : c0 + 1]
            # bf16 casts of W1/W2 for the Y-projection matmuls (h is bf16)
            w_sb = []
            for i, l in enumerate((1, 2)):
                wb = const.tile([DIMS[l], DIMS[l + 1]], BF16, name=f"w{l}bf")
                nc.vector.tensor_copy(out=wb[:], in_=(w1f, w2f)[i])
                w_sb.append(wb)

            # ---- CC mesh warm-up: tiny AllGather fired immediately, so the
            # first real collective doesn't pay the mesh cold cost ----
            wup_sb = smal.tile([2, 1], F32, name="wupsb")
            nc.vector.memset(wup_sb[:], 0.0)
            wup_in = dram.tile([2, 1], F32, tag="wui", name="wupin")
            wup_out = dram.tile(
                [NCORES, 2, 1], F32, tag="wuo", name="wupout",
                addr_space="Shared",
            )
            nc.gpsimd.dma_start(wup_in[:], wup_sb[:])
            nc.gpsimd.collective_compute(
                "AllGather",
                OP.bypass,
                replica_groups=rg,
                ins=[wup_in[:].opt()],
                outs=[wup_out[:].opt()],
            )

            a_cached = {}
            stat = None           # stat[piece] for current layer (l>=1)
            gpart = None          # mean-pool partials (layer 2)
            for l in range(3):
                c_stat = DIMS[l + 1]           # stationary width this layer
                c_next = DIMS[l + 2] if l < 2 else None
                h_sb = hpool.tile([c_stat, ROWS], BF16, tag="h", name=f"h{l}")
                stat_next = [None] * 4 if l < 2 else None
                if l == 2:
                    gpart = smal.tile([DIMS[3], 4], F32, name="gpart")
                for q in range(4):
                    iw = CW[q]
                    ioff = IOFF[q]
                    nt = NTQ[q]
                    acc = accp.tile(
                        [DIMS[3], iw], F32, tag=f"acc{q}", name=f"acc{l}_{q}"
                    )
                    for jg in range(NGQ[q]):
                        gp_all = (0, 16, 24, 28)[q] + jg
                        handle, gslot = group_meta(gp_all)
                        tshape = [P, nt, iw]
                        # alternate HWDGE rings (SP / ACT) for the stream
                        dma_eng = nc.sync if gp_all % 2 == 0 else nc.scalar
                        if gp_all in CACHED_IDS:
                            if l == 0:
                                a_sb = cpool.tile(
                                    tshape, F8, tag="c", name=f"ac{gp_all}"
                                )
                                dma_eng.dma_start(a_sb[:], handle.ap()[gslot])
                                a_cached[gp_all] = a_sb
                            else:
                                a_sb = a_cached[gp_all]
                        else:
                            a_sb = apool.tile(
                                tshape, F8, tag="a", name=f"a{l}_{gp_all}"
                            )
                            dma_eng.dma_start(a_sb[:], handle.ap()[gslot])
                        for t2 in range(nt // 2):
                            sp = jg * nt + 2 * t2
                            piece, r, u = _stripe_rv(sp)
                            if l == 0:
                                lw = stat0[:, r, u : u + 2, :]
                            else:
                                ul = u - (0, 8, 12, 14)[piece]
                                lw = stat[piece][:, r, ul : ul + 2, :]
                            first = jg == 0 and t2 == 0
                            last = jg == NGQ[q] - 1 and t2 == nt // 2 - 1
                            for ih in range(max(1, iw // 512)):
                                w512 = min(iw, 512)
                                nc.tensor.matmul(
                                    acc[:c_stat, ih * 512 : ih * 512 + w512],
                                    lhsT=lw,
                                    rhs=a_sb[
                                        :, 2 * t2 : 2 * t2 + 2,
                                        ih * 512 : ih * 512 + w512,
                                    ],
                                    start=first,
                                    stop=last,
                                    perf_mode=DR,
                                )

                    # ---- chunk q finalized: H columns [ioff, ioff+iw) ----
                    # high priority: the relu -> psy -> cast -> agin chain
                    # feeds the next layer's stationary AllGather; let it
                    # preempt queued stream MMs instead of smearing out.
                    with tc.high_priority():
                        nc.vector.tensor_scalar(
                            h_sb[:, ioff : ioff + iw],
                            acc[:c_stat, :iw],
                            b_sb[l][:], 0.0, OP.add, OP.max,
                        )

                    if l < 2:
                        # ---- project Y_{l+1} rows for this chunk's nodes,
                        #      AllGather as next layer's stationary piece ----
                        nu = NU[q]
                        y_sb = ypool.tile(
                            [P, nu, c_next], F8, tag="y", name=f"y{l}_{q}"
                        )
                        with tc.high_priority():
                            for u in range(nu):
                                n0 = ioff + u * P
                                ps = psml.tile(
                                    [P, c_next], F32, tag="psy",
                                    name=f"psy{l}_{q}_{u}",
                                )
                                nc.tensor.matmul(
                                    ps[:],
                                    lhsT=h_sb[:, n0 : n0 + P],
                                    rhs=w_sb[l][:],
                                    start=True,
                                    stop=True,
                                )
                                nc.vector.tensor_copy(
                                    out=y_sb[:, u, :], in_=ps[:]
                                )
                        ag_in = dram.tile(
                            [P, nu, c_next], F8, tag=f"agi{l}_{q}",
                            name=f"agin{l}_{q}",
                        )
                        ag_out = dram.tile(
                            [NCORES, P, nu, c_next], F8, tag=f"ago{l}_{q}",
                            name=f"agout{l}_{q}", addr_space="Shared",
                        )
                        nc.gpsimd.dma_start(ag_in[:], y_sb[:])
                        nc.gpsimd.collective_compute(
                            "AllGather",
                            OP.bypass,
                            replica_groups=rg,
                            ins=[ag_in[:].opt()],
                            outs=[ag_out[:].opt()],
                        )
                        st = spool.tile(
                            [P, NCORES, nu, c_next], F8, tag=f"st{q}",
                            name=f"st{l + 1}_{q}",
                        )
                        nc.scalar.dma_start(
                            st[:], ag_out[:].rearrange("r p u c -> p r u c")
                        )
                        stat_next[q] = st
                    else:
                        nc.vector.tensor_reduce(
                            gpart[:, q : q + 1], h_sb[:, ioff : ioff + iw],
                            axis=mybir.AxisListType.X, op=OP.add,
                        )
                stat = stat_next

            # ---- combine partials; AllGather 256B partials + local sum ----
            gp = smal.tile([DIMS[3], 1], F32, name="gpall")
            nc.vector.tensor_reduce(
                gp[:], gpart[:], axis=mybir.AxisListType.X, op=OP.add
            )
            ar_in = dram.tile([DIMS[3], 1], F32, tag="ari", name="arin")
            ar_out = dram.tile(
                [NCORES, DIMS[3], 1], F32, tag="aro", name="arout",
                addr_space="Shared",
            )
            nc.scalar.dma_start(ar_in[:], gp[:])
            nc.gpsimd.collective_compute(
                "AllGather",
                OP.bypass,
                replica_groups=rg,
                ins=[ar_in[:].opt()],
                outs=[ar_out[:].opt()],
            )
            g_all = smal.tile([DIMS[3], NCORES], F32, name="gall")
            nc.scalar.dma_start(g_all[:], ar_out[:].rearrange("r c o -> c (r o)"))
            g_sb = smal.tile([DIMS[3], 1], F32, name="gsb")
            nc.vector.tensor_reduce(
                g_sb[:], g_all[:], axis=mybir.AxisListType.X, op=OP.add
            )
            nc.any.tensor_scalar_mul(g_sb[:], g_sb[:], 1.0 / (N * ASCALE))

            # ---- MLP head: h1 = elu(g @ Wh1 + bh1) ----
            ps1 = psmlp.tile([32, 1], F32, tag="dum", name="ps1")
            nc.tensor.matmul(ps1[:], lhsT=wh1_sb[:], rhs=g_sb[:], start=True, stop=True)
            # elu(x) = relu(x) + exp(min(x, 0)) - 1
            tmin = smal.tile([32, 1], F32, name="tmin")
            nc.vector.tensor_scalar(tmin[:], ps1[:], bh1_sb[:], 0.0, OP.add, OP.min)
            e1 = smal.tile([32, 1], F32, name="e1")
            nc.scalar.activation(e1[:], tmin[:], AF.Exp)
            r1 = smal.tile([32, 1], F32, name="r1")
            nc.scalar.activation(r1[:], ps1[:], AF.Relu, bias=bh1_sb[:])
            h1 = smal.tile([32, 1], F32, name="h1")
            nc.vector.tensor_tensor(h1[:], e1[:], r1[:], OP.add)
            nc.vector.tensor_scalar_add(h1[:], h1[:], -1.0)

            # ---- logits = h1 @ Wh2 + bh2; probs = softmax(logits) ----
            ps2m = psmlp.tile([2, 1], F32, tag="dum", name="ps2m")
            nc.tensor.matmul(ps2m[:], lhsT=wh2_sb[:], rhs=h1[:], start=True, stop=True)
            logit_sb = smal.tile([2, 1], F32, name="logitsb")
            nc.vector.tensor_scalar(logit_sb[:], ps2m[:], bh2_sb[:], None, OP.add)
            nc.scalar.dma_start(logits_o.ap(), logit_sb[:])

            e2 = smal.tile([2, 1], F32, name="e2")
            nc.scalar.activation(e2[:], ps2m[:], AF.Exp, bias=bh2_sb[:])
            ones21 = smal.tile([2, 1], F32, name="ones21")
            nc.any.memset(ones21[:], 1.0)
            ones12 = smal.tile([1, 2], F32, name="ones12")
            nc.any.memset(ones12[:], 1.0)
            ps3 = psmlp.tile([1, 1], F32, tag="dum", name="ps3")
            nc.tensor.matmul(ps3[:], lhsT=e2[:], rhs=ones21[:], start=True, stop=True)
            rsc = smal.tile([1, 1], F32, name="rsc")
            nc.vector.reciprocal(rsc[:], ps3[:])
            ps4 = psmlp.tile([2, 1], F32, tag="dum", name="ps4")
            nc.tensor.matmul(ps4[:], lhsT=ones12[:], rhs=rsc[:], start=True, stop=True)
            probs_sb = smal.tile([2, 1], F32, name="probssb")
            nc.vector.tensor_tensor(probs_sb[:], e2[:], ps4[:], OP.mult)
            nc.scalar.dma_start(probs_o.ap(), probs_sb[:])

    nc.finalize()
    return nc


def _install_ntff_hook():
    """Register the axon NTFF profiling hook if the container's antenv stub
    lacks it (bass_utils imports antenv.axon_hooks when trace=True)."""
    import sys
    import types

    try:
        import antenv.axon_hooks  # noqa: F401
        return
    except ImportError:
        pass
    mod = types.ModuleType("antenv.axon_hooks")
    _h = [None]
    mod.set_axon_ntff_profile_hook = lambda h: _h.__setitem__(0, h)
    mod.get_axon_ntff_profile_hook = lambda: _h[0]
    sys.modules["antenv.axon_hooks"] = mod
    import antenv

    antenv.axon_hooks = mod
    try:
        from trn_agent_boot import trn_boot

        hook = trn_boot._ntff_profile_via_ctypes("/opt/axon/libaxon_pjrt.so")
        if hook is not None:
            mod.set_axon_ntff_profile_hook(hook)
    except Exception:
        pass


def _get_nc():
    global _nc_cache
    if _nc_cache is None:
        _nc_cache = _build_nc()
    return _nc_cache


_last_results = None


def _perm_rows():
    """j-row permutation matching _stripe_rv."""
    jidx = np.empty(N, dtype=np.int64)
    pos = 0
    ar = np.arange(P)
    for sp in range(128):
        piece, r, u = _stripe_rv(sp)
        jidx[pos : pos + P] = r * ROWS + u * P + ar
        pos += P
    return jidx


def kernel(
    node_feat,
    adj_matrix,
    W0,
    b0,
    W1,
    b1,
    W2,
    b2,
    Wh1,
    bh1,
    Wh2,
    bh2,
):
    global _last_results
    import os

    node_feat = np.ascontiguousarray(np.asarray(node_feat, dtype=np.float32))
    adj = np.asarray(adj_matrix, dtype=np.float32)

    # ---- host-side sharding / preprocessing ----
    deg = adj.sum(axis=1, dtype=np.float32) + 1.0
    dinv = (1.0 / np.sqrt(deg)).astype(np.float32)

    fp8 = ml_dtypes.float8_e4m3
    bf16 = ml_dtypes.bfloat16
    f32c = lambda a, shape=None: np.ascontiguousarray(
        np.asarray(a, dtype=np.float32).reshape(shape)
        if shape is not None
        else np.asarray(a, dtype=np.float32)
    )

    # layer-0 stationary = X @ W0 (natural scale), fp8,
    # layout [p, r, u, c]: node j = r*2048 + u*128 + p
    y0 = (node_feat @ np.asarray(W0, np.float32)).astype(fp8)
    s0 = np.ascontiguousarray(
        y0.reshape(NCORES, 16, P, DIMS[1]).transpose(2, 0, 1, 3)
    )

    # packed f32 const block: b0 | b1 | b2 | w1/AS | w2/AS | wh1 | bh1 | wh2 | bh2
    CPK_K = 3 + DIMS[2] + DIMS[3] + 32 + 1 + 2 + 1
    cpk = np.zeros((DIMS[0], CPK_K), dtype=np.float32)
    for l, b in enumerate((b0, b1, b2)):
        cpk[: DIMS[l + 1], l] = np.asarray(b, np.float32) * ASCALE
    c0 = 3
    cpk[: DIMS[1], c0 : c0 + DIMS[2]] = np.asarray(W1, np.float32) / ASCALE
    c0 += DIMS[2]
    cpk[: DIMS[2], c0 : c0 + DIMS[3]] = np.asarray(W2, np.float32) / ASCALE
    c0 += DIMS[3]
    cpk[: DIMS[3], c0 : c0 + 32] = np.asarray(Wh1, np.float32)
    c0 += 32
    cpk[:32, c0] = np.asarray(bh1, np.float32)
    c0 += 1
    cpk[:32, c0 : c0 + 2] = np.asarray(Wh2, np.float32)
    c0 += 2
    cpk[:2, c0] = np.asarray(bh2, np.float32)

    common = {"s0": s0, "cpk": cpk}

    jidx = _perm_rows()
    in_maps = []
    idx = np.arange(ROWS)
    sdinv = dinv * np.float32(ASCALE)
    for k in range(NCORES):
        sl = slice(k * ROWS, (k + 1) * ROWS)
        # rows of ASCALE*Ahat for this core's output nodes
        blk = adj[sl, :] * sdinv[sl, None]
        blk *= dinv[None, :]
        blk[idx, k * ROWS + idx] = sdinv[sl] * dinv[sl]  # + I self loops
        a_k = blk.T.astype(fp8)[jidx]  # [N, ROWS], j-permuted
        S = a_k.reshape(128, P, ROWS)  # [stripe, p, i]
        # chunk0: 16 groups [128, 8, 1024] (1MB)
        g0 = np.ascontiguousarray(
            S[:, :, : CW[0]]
            .reshape(NGQ[0], NTQ[0], P, CW[0]).transpose(0, 2, 1, 3)
        )
        # chunk1: 8 groups [128, 16, 512] (1MB)
        g1 = np.ascontiguousarray(
            S[:, :, IOFF[1] : IOFF[1] + CW[1]]
            .reshape(NGQ[1], NTQ[1], P, CW[1]).transpose(0, 2, 1, 3)
        )
        # chunks 2/3: 4+4 groups [128, 32, 256] (1MB)
        g2 = S[:, :, IOFF[2] : IOFF[2] + CW[2]].reshape(NGQ[2], NTQ[2], P, CW[2])
        g3 = S[:, :, IOFF[3] : IOFF[3] + CW[3]].reshape(NGQ[3], NTQ[3], P, CW[3])
        g23 = np.ascontiguousarray(
            np.concatenate([g2, g3], axis=0).transpose(0, 2, 1, 3)
        )
        m = {"a_c0": g0, "a_c1": g1, "a_c23": g23}
        m.update(common)
        in_maps.append(m)

    from concourse import bass_utils

    nc = _get_nc()
    trace = bool(int(os.environ.get("GCN_TRACE", "0")))
    if trace:
        _install_ntff_hook()
    res = bass_utils.run_bass_kernel_spmd(
        nc, in_maps, core_ids=list(range(NCORES)), trace=trace
    )
    _last_results = res

    out0 = res.results[0]
    logits = np.asarray(out0["logits"], dtype=np.float32).reshape(2)
    probs = np.asarray(out0["probs"], dtype=np.float32).reshape(2)
    return (logits, probs)
